# revision 16
# baseline (speedup 1.0000x reference)
"""Trainium2 Bass kernel for nn_Baseline_ResNet_LSTM (transformer greedy decoder).

Strategy:
  - Batch-parallel over 8 cores (8 batch elements/core) for the 3-layer
    transformer; vocab-parallel (3750 cols/core) for the 30k output projection.
  - One fixed-shape NEFF runs one decode step (padded to S=20 slots; the
    encoder has no positional encoding or mask, so token slots are
    permutation-invariant: slot 0 always holds the newest position and a
    host-supplied additive mask hides invalid slots).
  - Host loop (20 iterations): feeds embeddings, gets per-core logits shards,
    does softmax/argmax/feedback in numpy.  Weights stay device-resident.
  - fp32 matmuls everywhere (argmax margins as small as 7e-6 rule out bf16/fp16).
"""

import numpy as np

import concourse.bass as bass
import concourse.bacc as bacc
import concourse.mybir as mybir
import concourse.tile as tile
from concourse.bass import AP

F32 = mybir.dt.float32
AX = mybir.AxisListType
ALU = mybir.AluOpType
ACTF = mybir.ActivationFunctionType

D = 256
NHEAD = 4
HD = 64
FF = 2048
NLAYER = 3
VOCAB = 30000
BPTT = 20
EOS = 2
PAD = 0
NCORES = 8
BLOC = 8           # batch per core
T = BPTT * BLOC    # 160 padded tokens per core, t = s*8 + b
VSH = VOCAB // NCORES  # 3750 vocab cols per core
NEG = -1.0e9


def _bcast(ap_row, nparts):
    """Broadcast a [1, F] AP across nparts partitions (partition step 0)."""
    return ap_row[0, :].partition_broadcast(nparts)


def build_step_kernel():
    nc = bacc.Bacc(
        "TRN2", target_bir_lowering=False, debug=False, num_devices=NCORES
    )

    # ---- DRAM I/O ----
    d_emb = nc.dram_tensor("emb", [2, 128, T], F32, kind="ExternalInput")
    d_mask = nc.dram_tensor("mask", [T, T], F32, kind="ExternalInput")
    d_eye = nc.dram_tensor("eye", [128, 128], F32, kind="ExternalInput")
    d_wqkv = nc.dram_tensor("wqkv", [NLAYER, D, 3 * D], F32, kind="ExternalInput")
    d_wo = nc.dram_tensor("wo", [NLAYER, D, D], F32, kind="ExternalInput")
    d_w1 = nc.dram_tensor("w1", [NLAYER, D, FF], F32, kind="ExternalInput")
    d_w2 = nc.dram_tensor("w2", [NLAYER, FF, D], F32, kind="ExternalInput")
    # per-partition packed vectors: cols 0:4 bqkv(QK), 4:6 bo, 6:8 b2,
    # 8:10 g1, 10:12 be1, 12:14 g2, 14:16 be2, 16:32 b1
    d_pv = nc.dram_tensor("pv", [NLAYER, 128, 32], F32, kind="ExternalInput")
    d_brow = nc.dram_tensor("brow", [NLAYER, D], F32, kind="ExternalInput")  # bv rows
    d_wout = nc.dram_tensor("wout", [D, VSH], F32, kind="ExternalInput")
    F16 = mybir.dt.float16
    U32 = mybir.dt.uint32
    d_lg = nc.dram_tensor("logits", [64, VSH], F16, kind="ExternalOutput")
    d_mx = nc.dram_tensor("mx8", [8, 64, 8], F32, kind="ExternalOutput")
    d_ix = nc.dram_tensor("ix8", [8, 64, 8], U32, kind="ExternalOutput")

    # internal DRAM for the hidden-state AllGather
    d_hgin = nc.dram_tensor("hgin", [D, BLOC], F32, kind="Internal")
    d_hgout = nc.dram_tensor(
        "hgout", [NCORES * D, BLOC], F32, kind="Internal", addr_space="Shared"
    )

    with tile.TileContext(nc) as tc:
        with (
            tc.tile_pool(name="wpool", bufs=1) as wpool,
            tc.tile_pool(name="apool", bufs=1) as apool,
            tc.tile_pool(name="hpool", bufs=1) as hpool,
            tc.tile_pool(name="spool", bufs=1) as spool,
            tc.tile_pool(name="psp", bufs=1, space="PSUM") as psp,
        ):
            # ---- persistent constants / weights in SBUF ----
            def load(dram_ap, p, f, tag):
                t_ = wpool.tile([p, f], F32, tag=tag)
                nc.sync.dma_start(out=t_[:, :], in_=dram_ap)
                return t_

            X = [load(d_emb[c], 128, T, f"x{c}") for c in range(2)]
            maskA = load(d_mask.ap()[0:128, :], 128, T, "maskA")
            maskB = load(d_mask.ap()[128:T, :], 32, T, "maskB")
            eye = load(d_eye.ap()[:, :], 128, 128, "eye")

            wqkv = [[load(d_wqkv.ap()[l, 128 * k : 128 * (k + 1), :], 128, 3 * D,
                          f"wqkv{l}{k}") for k in range(2)] for l in range(NLAYER)]
            wo = [[load(d_wo.ap()[l, 128 * k : 128 * (k + 1), :], 128, D, f"wo{l}{k}")
                   for k in range(2)] for l in range(NLAYER)]
            w1 = [[load(d_w1.ap()[l, 128 * k : 128 * (k + 1), :], 128, FF, f"w1{l}{k}")
                   for k in range(2)] for l in range(NLAYER)]
            w2 = [[load(d_w2.ap()[l, 128 * k : 128 * (k + 1), :], 128, D, f"w2{l}{k}")
                   for k in range(16)] for l in range(NLAYER)]
            pv = [load(d_pv.ap()[l], 128, 32, f"pv{l}") for l in range(NLAYER)]
            brow = [load(d_brow.ap()[l : l + 1, :], 1, D, f"brow{l}") for l in range(NLAYER)]
            wout = [load(d_wout.ap()[128 * k : 128 * (k + 1), :], 128, VSH, f"wout{k}")
                    for k in range(2)]

            ones_row = spool.tile([1, T], F32, tag="ones_row")
            nc.vector.memset(ones_row[:, :], 1.0)
            ones_col = spool.tile([128, 1], F32, tag="ones_col")
            nc.gpsimd.memset(ones_col[:, :], 1.0)
            zcol = spool.tile([128, 1], F32, tag="zcol")
            nc.gpsimd.memset(zcol[:, :], 0.0)
            eps1 = spool.tile([1, 1], F32, tag="eps1")
            nc.vector.memset(eps1[:, :], 1e-5)

            # ---- transformer layers ----
            for l in range(NLAYER):
                # Q,K (feature-major [256,T] as 2x2 chunks) weights-stationary
                QKs = []
                for m in range(4):
                    ps = psp.tile([128, T], F32, tag="mm", bufs=2)
                    for k in range(2):
                        nc.tensor.matmul(
                            ps[:, :], wqkv[l][k][:, 128 * m : 128 * (m + 1)],
                            X[k][:, :], start=(k == 0), stop=(k == 1))
                    sb = apool.tile([128, T], F32, tag=f"qk{m}")
                    if m < 2:
                        nc.scalar.activation(sb[:, :], ps[:, :], ACTF.Identity,
                                             bias=pv[l][:, m : m + 1])
                    else:
                        nc.vector.tensor_scalar_add(sb[:, :], ps[:, :],
                                                    pv[l][:, m : m + 1])
                    QKs.append(sb)

                # V token-major [T, 256]: acts-stationary + ones-row bias
                Vs = []
                for mt, tw in ((0, 128), (1, 32)):
                    ps = psp.tile([128, D], F32, tag="mm", bufs=2)
                    for k in range(2):
                        nc.tensor.matmul(
                            ps[:tw, :], X[k][:, 128 * mt : 128 * mt + tw],
                            wqkv[l][k][:, 2 * D : 3 * D],
                            start=(k == 0), stop=False)
                    nc.tensor.matmul(ps[:tw, :], ones_row[:, :tw],
                                     brow[l][:, :], start=False, stop=True)
                    sb = apool.tile([128, D], F32, tag=f"v{mt}")
                    nc.scalar.activation(sb[:tw, :], ps[:tw, :], ACTF.Identity, bias=zcol[:tw, :])
                    Vs.append(sb)

                # attention per head
                Os = [apool.tile([128, T], F32, tag=f"o{c}", name=f"o{c}")
                      for c in range(2)]
                for h in range(4):
                    qt, row = QKs[h // 2], 64 * (h % 2)
                    kt = QKs[2 + h // 2]
                    Ph = []
                    for c, qw in ((0, 128), (1, 32)):
                        ps = psp.tile([128, T], F32, tag="att", bufs=2)
                        nc.tensor.matmul(
                            ps[:qw, :],
                            qt[row : row + 64, 128 * c : 128 * c + qw],
                            kt[row : row + 64, :], start=True, stop=True)
                        mk = maskA if c == 0 else maskB
                        nc.vector.tensor_add(ps[:qw, :], ps[:qw, :], mk[:qw, :])
                        mx = apool.tile([128, 1], F32, tag="mx", bufs=2)
                        nc.vector.reduce_max(mx[:qw, :], ps[:qw, :], axis=AX.X)
                        nm = apool.tile([128, 1], F32, tag="nm", bufs=2)
                        nc.vector.tensor_scalar_mul(nm[:qw, :], mx[:qw, :], -0.125)
                        z = apool.tile([128, 1], F32, tag="z", bufs=2)
                        pe = apool.tile([128, T], F32, tag="pe", bufs=2)
                        nc.scalar.activation(pe[:qw, :], ps[:qw, :], ACTF.Exp,
                                             bias=nm[:qw, :], scale=0.125,
                                             accum_out=z[:qw, :])
                        zi = apool.tile([128, 1], F32, tag="zi", bufs=2)
                        nc.vector.reciprocal(zi[:qw, :], z[:qw, :])
                        pn = apool.tile([128, T], F32, tag="pn", bufs=2)
                        nc.vector.tensor_scalar_mul(pn[:qw, :], pe[:qw, :],
                                                    zi[:qw, :])
                        Ph.append((pn, qw))
                    # transpose P -> PT [t_k, t_q]
                    PTs = []
                    for b, kw in ((0, 128), (1, 32)):
                        pst = psp.tile([128, T], F32, tag="ptp")
                        for a, qw in ((0, 128), (1, 32)):
                            nc.tensor.transpose(
                                pst[:kw, 128 * a : 128 * a + qw],
                                Ph[a][0][:qw, 128 * b : 128 * b + kw],
                                eye[:qw, :qw])
                        sb = apool.tile([128, T], F32, tag=f"pt{b}", bufs=2)
                        nc.scalar.activation(sb[:kw, :], pst[:kw, :], ACTF.Identity,
                                             bias=zcol[:kw, :])
                        PTs.append((sb, kw))
                    pso = psp.tile([128, T], F32, tag="pso")
                    for b, kw in ((0, 128), (1, 32)):
                        nc.tensor.matmul(
                            pso[:64, :], Vs[b][:kw, 64 * h : 64 * h + 64],
                            PTs[b][0][:kw, :], start=(b == 0), stop=(b == 1))
                    nc.scalar.activation(Os[h // 2][row : row + 64, :],
                                         pso[:64, :], ACTF.Identity,
                                         bias=zcol[:64, :])

                # Wo + residual + LN1
                Xn = []
                R32s = []
                for m in range(2):
                    ps = psp.tile([128, T], F32, tag="mm", bufs=2)
                    for k in range(2):
                        nc.tensor.matmul(ps[:, :],
                                         wo[l][k][:, 128 * m : 128 * (m + 1)],
                                         Os[k][:, :], start=(k == 0), stop=(k == 1))
                    r = apool.tile([128, T], F32, tag=f"r{m}")
                    nc.vector.scalar_tensor_tensor(
                        r[:, :], ps[:, :], pv[l][:, 4 + m : 5 + m], X[m][:, :],
                        op0=ALU.add, op1=ALU.add)
                    R32s.append(r)
                Xn = _layernorm(nc, tc, apool, psp, R32s, ones_col,
                                pv[l], 8, 10, T, zcol, eps1)

                # FFN
                Hs = []
                for m in range(16):
                    ps = psp.tile([128, T], F32, tag="mm", bufs=2)
                    for k in range(2):
                        nc.tensor.matmul(ps[:, :],
                                         w1[l][k][:, 128 * m : 128 * (m + 1)],
                                         Xn[k][:, :], start=(k == 0), stop=(k == 1))
                    hsb = hpool.tile([128, T], F32, tag=f"h{m}")
                    if m % 2 == 0:
                        nc.scalar.activation(hsb[:, :], ps[:, :], ACTF.Relu,
                                             bias=pv[l][:, 16 + m : 17 + m])
                    else:
                        nc.vector.tensor_scalar(hsb[:, :], ps[:, :],
                                                pv[l][:, 16 + m : 17 + m], 0.0,
                                                op0=ALU.add, op1=ALU.max)
                    Hs.append(hsb)
                R32b = []
                for m in range(2):
                    ps = psp.tile([128, T], F32, tag="mm", bufs=2)
                    for k in range(16):
                        nc.tensor.matmul(ps[:, :],
                                         w2[l][k][:, 128 * m : 128 * (m + 1)],
                                         Hs[k][:, :], start=(k == 0),
                                         stop=(k == 15))
                    r = apool.tile([128, T], F32, tag=f"rb{m}")
                    nc.vector.scalar_tensor_tensor(
                        r[:, :], ps[:, :], pv[l][:, 6 + m : 7 + m], Xn[m][:, :],
                        op0=ALU.add, op1=ALU.add)
                    R32b.append(r)
                X = _layernorm(nc, tc, apool, psp, R32b, ones_col,
                               pv[l], 12, 14, T, zcol, eps1)

            # ---- gather hidden states of slot 0 across cores ----
            for c in range(2):
                nc.sync.dma_start(out=d_hgin.ap()[128 * c : 128 * (c + 1), :],
                                  in_=X[c][:, 0:BLOC])
            nc.gpsimd.collective_compute(
                "AllGather", ALU.bypass,
                replica_groups=[list(range(NCORES))],
                ins=[d_hgin.ap()], outs=[d_hgout.ap()])

            Hh = []
            hg = d_hgout.ap().rearrange("(r two p) b -> two p r b", r=NCORES,
                                        two=2, p=128)
            for c in range(2):
                t_ = spool.tile([128, 64], F32, tag=f"hh{c}")
                nc.sync.dma_start(
                    out=t_[:, :].rearrange("p (r b) -> p r b", r=NCORES), in_=hg[c])
                Hh.append(t_)

            # ---- logits [64, VSH] ----
            nchunks = (VSH + 511) // 512
            for n in range(nchunks):
                n0 = 512 * n
                w = min(512, VSH - n0)
                ps = psp.tile([64, 512], F32, tag="mm", bufs=2)
                for k in range(2):
                    nc.tensor.matmul(ps[:, :w], Hh[k][:, :],
                                     wout[k][:, n0 : n0 + w],
                                     start=(k == 0), stop=(k == 1))
                sb = apool.tile([64, 512], F32, tag="lgsb", bufs=2)
                if n % 2 == 0:
                    nc.scalar.activation(sb[:, :w], ps[:, :w], ACTF.Identity,
                                         bias=zcol[:64, :])
                else:
                    nc.vector.tensor_copy(sb[:, :w], ps[:, :w])
                mx8 = apool.tile([64, 8], F32, tag="mx8", bufs=2)
                ix8 = apool.tile([64, 8], U32, tag="ix8", bufs=2)
                nc.vector.max_with_indices(mx8[:, :], ix8[:, :], sb[:, :w])
                sb16 = apool.tile([64, 512], F16, tag="lg16", bufs=2)
                nc.gpsimd.tensor_copy(sb16[:, :w], sb[:, :w])
                nc.sync.dma_start(out=d_lg.ap()[:, n0 : n0 + w], in_=sb16[:, :w])
                nc.sync.dma_start(out=d_mx.ap()[n], in_=mx8[:, :])
                nc.sync.dma_start(out=d_ix.ap()[n], in_=ix8[:, :])

    nc.compile()
    return nc


def _layernorm(nc, tc, apool, psp, R, ones_col, pvt, gcol, bcol, T, zcol, eps1):
    """fm layernorm over partitions (2 chunks x [128,T]) -> new fm X chunks."""
    ACTF = mybir.ActivationFunctionType
    ALU = mybir.AluOpType
    AX = mybir.AxisListType
    R2 = []
    for c in range(2):
        r2 = apool.tile([128, T], F32, tag=f"lnsq{c}")
        nc.scalar.activation(r2[:, :], R[c][:, :], ACTF.Square, bias=zcol[:, :])
        R2.append(r2)
    ps1 = psp.tile([1, T], F32, tag="lns1")
    ps2 = psp.tile([1, T], F32, tag="lns2")
    for k in range(2):
        nc.tensor.matmul(ps1[:, :], ones_col[:, :], R[k][:, :],
                         start=(k == 0), stop=(k == 1))
    for k in range(2):
        nc.tensor.matmul(ps2[:, :], ones_col[:, :], R2[k][:, :],
                         start=(k == 0), stop=(k == 1))
    mu = apool.tile([1, T], F32, tag="ln_mu")
    nc.vector.tensor_scalar_mul(mu[:, :], ps1[:, :], 1.0 / 256.0)
    ms = apool.tile([1, T], F32, tag="ln_ms")
    nc.vector.tensor_mul(ms[:, :], mu[:, :], mu[:, :])
    v = apool.tile([1, T], F32, tag="ln_v")
    nc.vector.scalar_tensor_tensor(v[:, :], ps2[:, :], 1.0 / 256.0, ms[:, :],
                                   op0=ALU.mult, op1=ALU.subtract)
    sd = apool.tile([1, T], F32, tag="ln_sd")
    nc.scalar.activation(sd[:, :], v[:, :], ACTF.Sqrt, bias=eps1[:, :])
    r0 = apool.tile([1, T], F32, tag="ln_r0")
    nc.vector.reciprocal(r0[:, :], sd[:, :])
    # one Newton step: r1 = r0*(1.5 - 0.5*(v+eps)*r0^2)
    vh = apool.tile([1, T], F32, tag="ln_vh")
    nc.vector.tensor_scalar_add(vh[:, :], v[:, :], 1e-5)
    t1 = apool.tile([1, T], F32, tag="ln_t1")
    nc.vector.tensor_mul(t1[:, :], r0[:, :], r0[:, :])
    nc.vector.tensor_mul(t1[:, :], t1[:, :], vh[:, :])
    nc.vector.tensor_scalar(t1[:, :], t1[:, :], -0.5, 1.5,
                            op0=ALU.mult, op1=ALU.add)
    r1 = apool.tile([1, T], F32, tag="ln_r1")
    nc.vector.tensor_mul(r1[:, :], r0[:, :], t1[:, :])
    c_ = apool.tile([1, T], F32, tag="ln_c")
    nc.vector.tensor_mul(c_[:, :], mu[:, :], r1[:, :])
    rb = apool.tile([128, T], F32, tag="ln_rb")
    nc.gpsimd.partition_broadcast(rb[:, :], r1[0:1, :])
    cb = apool.tile([128, T], F32, tag="ln_cb")
    nc.gpsimd.partition_broadcast(cb[:, :], c_[0:1, :])
    out = []
    for c in range(2):
        y = apool.tile([128, T], F32, tag=f"ln_y{c}")
        nc.vector.tensor_mul(y[:, :], R[c][:, :], rb[:, :])
        nc.vector.tensor_sub(y[:, :], y[:, :], cb[:, :])
        xo = apool.tile([128, T], F32, tag=f"ln_x{c}")
        nc.vector.tensor_scalar(xo[:, :], y[:, :],
                                pvt[:, gcol + c : gcol + c + 1],
                                pvt[:, bcol + c : bcol + c + 1],
                                op0=ALU.mult, op1=ALU.add)
        out.append(xo)
    return out


# ----------------------------------------------------------------------------
# host side
# ----------------------------------------------------------------------------

_CACHE = {}


def _install_neff_disk_cache():
    import hashlib, os
    from concourse import bass2jax, bass_utils

    if getattr(bass_utils, "_ant_neff_cache_installed", False):
        return
    cdir = os.environ.get("BASS_NEFF_CACHE_DIR", "/tmp/bass_neff_cache")
    os.makedirs(cdir, exist_ok=True)
    orig = bass_utils.compile_bir_kernel

    def cached(bir_json, tmpdir, neff_name="file.neff"):
        key = hashlib.sha256(bir_json).hexdigest()[:32]
        path = os.path.join(cdir, f"{key}_{neff_name}")
        if os.path.exists(path):
            dst = os.path.join(tmpdir, neff_name)
            with open(path, "rb") as f, open(dst, "wb") as g:
                g.write(f.read())
            return dst
        out = orig(bir_json, tmpdir, neff_name=neff_name)
        try:
            with open(out, "rb") as f, open(path, "wb") as g:
                g.write(f.read())
        except OSError:
            pass
        return out

    bass_utils.compile_bir_kernel = cached
    if getattr(bass2jax, "compile_bir_kernel", None) is not None:
        bass2jax.compile_bir_kernel = cached
    bass_utils._ant_neff_cache_installed = True


def _get_runner():
    if "runner" in _CACHE:
        return _CACHE["runner"]
    import jax
    from concourse import bass2jax
    from concourse.bass2jax import _bass_exec_p
    from jax.sharding import Mesh, PartitionSpec
    from jax.experimental.shard_map import shard_map

    _install_neff_disk_cache()
    nc = build_step_kernel()
    bass2jax.install_neuronx_cc_hook()

    partition_name = (nc.partition_id_tensor.name
                      if nc.partition_id_tensor else None)
    in_names, out_names, out_avals, zero_outs = [], [], [], []
    for alloc in nc.m.functions[0].allocations:
        if not isinstance(alloc, mybir.MemoryLocationSet):
            continue
        name = alloc.memorylocations[0].name
        if alloc.kind == "ExternalInput":
            if name != partition_name:
                in_names.append(name)
        elif alloc.kind == "ExternalOutput":
            shape = tuple(alloc.tensor_shape)
            dtype = mybir.dt.np(alloc.dtype)
            out_names.append(name)
            out_avals.append(jax.core.ShapedArray(shape, dtype))
            zero_outs.append(np.zeros(shape, dtype))
    n_params = len(in_names)
    all_names = in_names + out_names
    if partition_name is not None:
        all_names = all_names + [partition_name]

    def _body(*args):
        operands = list(args)
        if partition_name is not None:
            operands.append(bass2jax.partition_id_tensor())
        outs = _bass_exec_p.bind(
            *operands,
            out_avals=tuple(out_avals),
            in_names=tuple(all_names),
            out_names=tuple(out_names),
            lowering_input_output_aliases=(),
            sim_require_finite=False,
            sim_require_nnan=False,
            nc=nc,
        )
        return tuple(outs)

    devices = jax.devices()[:NCORES]
    mesh = Mesh(np.asarray(devices), ("core",))
    n_outs = len(out_names)
    sharded = jax.jit(
        shard_map(
            _body, mesh=mesh,
            in_specs=(PartitionSpec("core"),) * (n_params + n_outs),
            out_specs=(PartitionSpec("core"),) * n_outs,
            check_rep=False,
        ),
        donate_argnums=tuple(range(n_params, n_params + n_outs)),
        keep_unused=True,
    )
    runner = (nc, sharded, in_names, out_names, zero_outs, jax, mesh)
    _CACHE["runner"] = runner
    return runner


def _np(x):
    return np.ascontiguousarray(np.asarray(x, dtype=np.float32))


def kernel(image_feats, target_ingrs, emb_table, W_g2e, b_g2e, W_out, b_out,
           Wqkv, bqkv, Wo, bo, W1, b1, W2, b2, g1, be1, g2, be2):
    nc, sharded, in_names, out_names, zero_outs, jax, mesh = _get_runner()

    image_feats = _np(image_feats)
    emb_table = _np(emb_table)
    W_out = _np(W_out)
    b_out_np = _np(b_out)

    # per-partition packed vectors [3, 128, 32]
    pv = np.zeros((NLAYER, 128, 32), np.float32)
    bqkv_n, bo_n, b1_n, b2_n = _np(bqkv), _np(bo), _np(b1), _np(b2)
    g1_n, be1_n, g2_n, be2_n = _np(g1), _np(be1), _np(g2), _np(be2)
    for l in range(NLAYER):
        pv[l, :, 0:4] = bqkv_n[l][: 2 * D].reshape(4, 128).T
        pv[l, :, 4:6] = bo_n[l].reshape(2, 128).T
        pv[l, :, 6:8] = b2_n[l].reshape(2, 128).T
        pv[l, :, 8:10] = g1_n[l].reshape(2, 128).T
        pv[l, :, 10:12] = be1_n[l].reshape(2, 128).T
        pv[l, :, 12:14] = g2_n[l].reshape(2, 128).T
        pv[l, :, 14:16] = be2_n[l].reshape(2, 128).T
        pv[l, :, 16:32] = b1_n[l].reshape(16, 128).T
    brow = bqkv_n[:, 2 * D :]  # [3, 256]
    eye = np.eye(128, dtype=np.float32)

    # static per-core device arrays (uploaded once)
    static = {
        "eye": np.broadcast_to(eye, (NCORES, 128, 128)),
        "wqkv": np.broadcast_to(_np(Wqkv), (NCORES, NLAYER, D, 3 * D)),
        "wo": np.broadcast_to(_np(Wo), (NCORES, NLAYER, D, D)),
        "w1": np.broadcast_to(_np(W1), (NCORES, NLAYER, D, FF)),
        "w2": np.broadcast_to(_np(W2), (NCORES, NLAYER, FF, D)),
        "pv": np.broadcast_to(pv, (NCORES, NLAYER, 128, 32)),
        "brow": np.broadcast_to(brow, (NCORES, NLAYER, D)),
        "wout": np.stack([W_out[:, VSH * c : VSH * (c + 1)] for c in range(NCORES)]),
    }
    # concat over cores on axis 0 (shard_map shards axis 0); upload once
    from jax.sharding import NamedSharding, PartitionSpec as P
    sh = NamedSharding(mesh, P("core"))
    devs = {}
    for k, v in static.items():
        a = np.ascontiguousarray(v).reshape((v.shape[0] * v.shape[1],) + v.shape[2:])
        devs[k] = jax.device_put(a, sh)

    import jax.numpy as jnp
    zshapes = [(NCORES * z.shape[0],) + z.shape[1:] for z in zero_outs]
    zdtypes = [z.dtype for z in zero_outs]
    zmaker = jax.jit(
        lambda: tuple(jnp.zeros(s, d) for s, d in zip(zshapes, zdtypes)),
        out_shardings=tuple(sh for _ in zshapes))

    # greedy decode
    img_embed = image_feats @ _np(W_g2e) + _np(b_g2e)  # [64, 256]
    slots = np.zeros((BPTT, 64, D), np.float32)
    slots[0] = img_embed

    base_mask = np.full((T, T), NEG, np.float32)
    tt = np.arange(T)
    same_b = (tt[:, None] % BLOC) == (tt[None, :] % BLOC)

    probs_run = np.zeros((64, VOCAB), np.float32)
    words = np.zeros((64, BPTT), np.int64)
    eos_out = np.zeros((64, BPTT), np.float32)
    mask_steps = np.asarray(
        np.cumprod(
            np.concatenate(
                [np.ones((64, 1), np.int32),
                 (np.asarray(target_ingrs)[:, 1:] != EOS).astype(np.int32)],
                axis=1),
            axis=1))

    import time as _time
    _CACHE["step_times"] = []
    pending = []
    b_out_nonzero = bool(np.any(b_out_np))

    def deferred():
        nonlocal probs_run
        while pending:
            j, lgt = pending.pop(0)
            mxv = lgt.max(axis=1, keepdims=True)
            e = np.exp(lgt - mxv)
            smv = e / e.sum(axis=1, keepdims=True)
            eos_out[:, j] = smv[:, EOS]
            probs_run = np.maximum(
                probs_run, smv * mask_steps[:, j : j + 1].astype(np.float32))

    masks_dev = []
    for i in range(BPTT):
        m = base_mask.copy()
        valid = same_b & ((tt[None, :] // BLOC) <= i)
        m[valid] = 0.0
        masks_dev.append(jax.device_put(
            np.ascontiguousarray(
                np.broadcast_to(m, (NCORES,) + m.shape)).reshape(NCORES * T, T),
            sh))

    for i in range(BPTT):
        _t0 = _time.time()

        emb = np.zeros((NCORES, 2, 128, T), np.float32)
        # emb_fm[d, s*8+b] = slots[s, 8c+b, d]
        sl = slots.transpose(2, 0, 1)  # [256, 20, 64]
        for c in range(NCORES):
            blk = sl[:, :, BLOC * c : BLOC * (c + 1)].reshape(D, T)
            emb[c, 0] = blk[:128]
            emb[c, 1] = blk[128:]

        feed = dict(devs)
        feed["emb"] = jax.device_put(emb.reshape(NCORES * 2, 128, T), sh)
        feed["mask"] = masks_dev[i]
        args = [feed[n] for n in in_names] + list(zmaker())
        outs = sharded(*args)
        deferred()
        mxs = np.asarray(outs[out_names.index("mx8")]).reshape(
            NCORES, 8, 64, 8)[:, :, :, 0]          # [core, chunk, b]
        ixs = np.asarray(outs[out_names.index("ix8")]).reshape(
            NCORES, 8, 64, 8)[:, :, :, 0].astype(np.int64)
        _CACHE["step_times"].append(_time.time() - _t0)
        # exact argmax from per-chunk fp32 maxima (vocab order = (core, chunk))
        flat_mx = mxs.transpose(2, 0, 1).reshape(64, NCORES * 8)
        flat_ix = (ixs
                   + (np.arange(8) * 512)[None, :, None]
                   + (np.arange(NCORES) * VSH)[:, None, None]
                   ).transpose(2, 0, 1).reshape(64, NCORES * 8)
        best = flat_mx.argmax(axis=1)
        widx = flat_ix[np.arange(64), best]
        lg = np.asarray(outs[out_names.index("logits")]).reshape(
            NCORES, 64, VSH)
        logits = np.concatenate(
            [lg[c] for c in range(NCORES)], axis=1).astype(np.float32)
        logits = logits + b_out_np[None, :]
        if b_out_nonzero:
            # device stats exclude b_out; fall back to host argmax
            widx = logits.argmax(axis=1)
        words[:, i] = widx
        pending.append((i, logits))
        if i + 1 < BPTT:
            # shift: slot j holds position i+1-j
            slots[1 : i + 2] = slots[0 : i + 1].copy()
            slots[0] = emb_table[widx]
        else:
            deferred()

    sampled = np.where(mask_steps == 0, PAD, words).astype(np.int32)
    return probs_run, sampled, eos_out.astype(np.float32)
